# revision 1
# baseline (speedup 1.0000x reference)
"""Multi-head causal self-attention (B=4, S=2048, D=1024, H=16) on 8 TRN2 NeuronCores.

Sharding: core c handles batch b = c//2 and head-group g = c%2 (heads 8g..8g+7).
Per core:
  - QKV projections with column-sharded weights (bf16 matmuls, f32 accum)
  - causal attention for its 8 heads with transposed score tiles S_T[sk, sq];
    softmax denominators come from an augmented-V matmul (ones column per head),
    the causal mask is a multiplicative 0/1 mask applied to the exp'd tiles,
    and 1/denominator runs on the scalar engine's Reciprocal table
  - per-pair AllGather of the attention output (outT), then the output
    projection with the core's 512-column slice of Wo (column-split y)
Host: transposes/casts inputs to bf16, assembles y from per-core column halves.
"""

import numpy as np
import ml_dtypes

import concourse.bass as bass
import concourse.tile as tile
from concourse import bacc, mybir
from concourse.bass_utils import run_bass_kernel_spmd

BF16 = ml_dtypes.bfloat16
N_CORES = 8
B, S, D, H = 4, 2048, 1024, 16
HD = D // H          # 64 head dim
HL = H // 2          # 8 local heads
DL = D // 2          # 512 local d (= HL * HD), also the y column half

_PROGRAM_CACHE = {}
_LAST_IN_MAPS = None


def _act_reciprocal(nc, out_ap, in_ap):
    """ScalarE Reciprocal via raw InstActivation (bass wrapper refuses it;
    measured max rel err 1.2e-5 on HW, fine at our tolerance)."""
    eng = nc.scalar
    inputs = [eng.lower_ap(in_ap)]
    for val in (0.0, 1.0, 0.0):  # bias, scale, alpha
        inputs.append(mybir.ImmediateValue(dtype=mybir.dt.float32, value=val))
    return eng.add_instruction(
        mybir.InstActivation(
            name=nc.get_next_instruction_name(),
            func=mybir.ActivationFunctionType.Reciprocal,
            ins=inputs,
            outs=[eng.lower_ap(out_ap)],
        )
    )


def _build_program(with_bias):
    bf = mybir.dt.bfloat16
    f32 = mybir.dt.float32
    FT = mybir.ActivationFunctionType
    ALU = mybir.AluOpType

    nc = bacc.Bacc(None)
    xT_d = nc.declare_dram_parameter("xT", [D, S], bf, isOutput=False)
    wq_d = nc.declare_dram_parameter("wqT", [D, DL], bf, isOutput=False)
    wk_d = nc.declare_dram_parameter("wkT", [D, DL], bf, isOutput=False)
    wv_d = nc.declare_dram_parameter("wvT", [D, DL], bf, isOutput=False)
    wo_d = nc.declare_dram_parameter("woT", [D, DL], bf, isOutput=False)
    mm_d = nc.declare_dram_parameter("mmask", [4, 128, 512], bf, isOutput=False)
    if with_bias:
        bq_d = nc.declare_dram_parameter("bq", [1, DL], bf, isOutput=False)
        bk_d = nc.declare_dram_parameter("bk", [1, DL], bf, isOutput=False)
        bv_d = nc.declare_dram_parameter("bv", [1, DL], bf, isOutput=False)
        bo_d = nc.declare_dram_parameter("bo", [1, DL], bf, isOutput=False)
    out_d = nc.declare_dram_parameter("out", [S, DL], f32, isOutput=True)

    groups = [[0, 1], [2, 3], [4, 5], [6, 7]]

    with tile.TileContext(nc) as tc:
        with (
            tc.tile_pool(name="const", bufs=1) as cpool,
            tc.tile_pool(name="acts", bufs=1) as apool,
            tc.tile_pool(name="attn", bufs=4) as atpool,
            tc.tile_pool(name="onorm", bufs=4) as opool,
            tc.tile_pool(name="rc", bufs=2) as rcpool,
            tc.tile_pool(name="bc", bufs=3) as bcpool,
            tc.tile_pool(name="woin", bufs=12) as wipool,
            tc.tile_pool(name="ysb", bufs=3) as ypool,
            tc.tile_pool(name="psS", bufs=2, space="PSUM") as psS,
            tc.tile_pool(name="psA", bufs=3, space="PSUM") as psA,
            tc.tile_pool(name="psY", bufs=1, space="PSUM") as psY,
            tc.tile_pool(name="dram", bufs=1, space="DRAM") as dpool,
        ):
            # ---- load weights / constants --------------------------------
            xt = [cpool.tile([128, S], bf, name=f"xt{k}", tag=f"xt{k}") for k in range(8)]
            wq = [cpool.tile([128, DL], bf, name=f"wq{k}", tag=f"wq{k}") for k in range(8)]
            wk = [cpool.tile([128, DL], bf, name=f"wk{k}", tag=f"wk{k}") for k in range(8)]
            wv = [cpool.tile([128, DL], bf, name=f"wv{k}", tag=f"wv{k}") for k in range(8)]
            wo = [cpool.tile([128, DL], bf, name=f"wo{k}", tag=f"wo{k}") for k in range(8)]
            msk = [cpool.tile([128, 2, 512], bf, name=f"msk{c}", tag=f"msk{c}") for c in range(2)]
            # load order matters at kernel start: x and Q/K weights gate the
            # first matmuls; Wo isn't needed until the first exchange
            for k in range(8):
                nc.scalar.dma_start(xt[k][:], xT_d[128 * k:128 * k + 128, :])
                nc.scalar.dma_start(wq[k][:], wq_d[128 * k:128 * k + 128, :])
                nc.scalar.dma_start(wk[k][:], wk_d[128 * k:128 * k + 128, :])
            for k in range(8):
                nc.scalar.dma_start(wv[k][:], wv_d[128 * k:128 * k + 128, :])
            for c in range(4):
                nc.scalar.dma_start(msk[c // 2][:, c % 2, :], mm_d[c])
            for k in range(8):
                nc.scalar.dma_start(wo[k][:], wo_d[128 * k:128 * k + 128, :])
            if with_bias:
                ones = cpool.tile([1, 512], bf, tag="ones")
                nc.vector.memset(ones[:], 1.0)
                bq = cpool.tile([1, DL], bf, tag="bq")
                bk = cpool.tile([1, DL], bf, tag="bk")
                bv = cpool.tile([1, DL], bf, tag="bv")
                bo = cpool.tile([1, DL], bf, tag="bo")
                nc.gpsimd.dma_start(bq[:], bq_d[:])
                nc.gpsimd.dma_start(bk[:], bk_d[:])
                nc.gpsimd.dma_start(bv[:], bv_d[:])
                nc.gpsimd.dma_start(bo[:], bo_d[:])

            # ---- phase 1: QKV projections --------------------------------
            # Q_T/K_T: [d_out_local, s] in 4 pair tiles of [128, S]
            qt = [apool.tile([128, S], bf, name=f"qt{m}", tag=f"qt{m}") for m in range(4)]
            kt = [apool.tile([128, S], bf, name=f"kt{m}", tag=f"kt{m}") for m in range(4)]
            # V: [s, d_out_local] padded with a ones column per head
            vt = [apool.tile([128, HL * (HD + 1)], bf, name=f"v{s}", tag=f"v{s}") for s in range(16)]

            # weight-stationary: 4 s-chunks accumulate concurrently so each
            # lhsT (weight tile) is loaded once per k instead of once per mm
            for wtiles, bname, dst in ((wq, "bq", qt), (wk, "bk", kt)):
                for m in range(4):
                    ps4 = [psY.tile([128, 512], f32, name=f"psqk{s4}", tag="ps_y")
                           if s4 < 1 else
                           psA.tile([128, 512], f32, name=f"psqk{s4}", tag="ps_a")
                           for s4 in range(4)]
                    for k in range(8):
                        for s4 in range(4):
                            nc.tensor.matmul(
                                ps4[s4][:], wtiles[k][:, 128 * m:128 * m + 128],
                                xt[k][:, 512 * s4:512 * s4 + 512],
                                start=(k == 0),
                                stop=(k == 7 and not with_bias),
                            )
                    for s4 in range(4):
                        if with_bias:
                            bt = bq if bname == "bq" else bk
                            nc.tensor.matmul(
                                ps4[s4][:], bt[0:1, 128 * m:128 * m + 128],
                                ones[0:1, :], start=False, stop=True,
                            )
                        nc.vector.tensor_copy(dst[m][:, 512 * s4:512 * s4 + 512],
                                              ps4[s4][:])

            for s in range(16):
                pool_ = psY if s % 2 == 0 else psA
                tag_ = "ps_y" if s % 2 == 0 else "ps_a"
                ps = pool_.tile([128, 512], f32, name="psv", tag=tag_)
                for k in range(8):
                    nc.tensor.matmul(
                        ps[:], xt[k][:, 128 * s:128 * s + 128], wv[k][:],
                        start=(k == 0), stop=(k == 7 and not with_bias),
                    )
                if with_bias:
                    nc.tensor.matmul(ps[:], ones[0:1, 0:128], bv[0:1, :],
                                     start=False, stop=True)
                vv = vt[s][:].rearrange("p (h x) -> p h x", x=HD + 1)
                nc.vector.tensor_copy(
                    vv[:, :, 0:HD],
                    ps[:].rearrange("p (h x) -> p h x", x=HD),
                )
                nc.vector.memset(vv[:, :, HD:HD + 1], 1.0)

            # ---- phase 2: attention + chunked exchange + out-proj --------
            ag_in = [dpool.tile([DL, 512], bf, name=f"agin{q}", tag=f"agin{q}")
                     for q in range(4)]
            ag_out = [dpool.tile([2, DL, 512], bf, name=f"agout{q}",
                                 tag=f"agout{q}") for q in range(4)]

            for q in range(4):
                n_sk = 4 * (q + 1)
                # denominators parked in 32-aligned rows (partition-shifted
                # writes must start at a 32-aligned partition)
                den = [rcpool.tile([128, 512], f32, name=f"den{j}", tag=f"den{j}")
                       for j in range(2)]
                ou = []
                for p in range(4):
                    # both heads of a pair advance together: score matmuls
                    # alternate PE row groups (base partition 0/64), hiding
                    # LDWEIGHTS behind the other head's matmul
                    av = [psA.tile([128, 512], f32, name=f"av{sub}", tag="ps_a")
                          for sub in range(2)]
                    for gi in range(n_sk // 2):
                        ats = []
                        for sub in range(2):
                            po = 64 * sub
                            sc = psS.tile([128, 2, 512], f32, name="sc", tag="ps_s")
                            for c2 in range(2):
                                ci = 2 * gi + c2
                                nc.tensor.matmul(
                                    sc[:, c2, :],
                                    kt[p][po:po + 64, 128 * ci:128 * ci + 128],
                                    qt[p][po:po + 64, 512 * q:512 * q + 512],
                                    start=True, stop=True,
                                )
                            at = atpool.tile([128, 2, 512], bf, name="at", tag="at")
                            nc.scalar.activation(at[:], sc[:], FT.Exp, scale=0.125)
                            if 2 * gi >= 4 * q:  # diagonal group: causal mask
                                atm = atpool.tile([128, 2, 512], bf, name="atm",
                                                  tag="at")
                                nc.vector.tensor_tensor(
                                    atm[:], at[:], msk[gi - 2 * q][:], op=ALU.mult)
                                at = atm
                            ats.append(at)
                        for sub in range(2):
                            h = 2 * p + sub
                            for c2 in range(2):
                                ci = 2 * gi + c2
                                nc.tensor.matmul(
                                    av[sub][0:HD + 1, :],
                                    vt[ci][:, (HD + 1) * h:(HD + 1) * h + HD + 1],
                                    ats[sub][:, c2, :],
                                    start=(ci == 0), stop=(ci == n_sk - 1),
                                )
                    # stage unnormalized out + denominator to SBUF, free av
                    for sub in range(2):
                        h = 2 * p + sub
                        o65 = opool.tile([65, 512], f32, name="o65", tag="o65",
                                         bufs=9)
                        nc.scalar.copy(o65[:], av[sub][0:65, :])
                        r = 32 * (h % 4)
                        nc.vector.tensor_copy(den[h // 4][r:r + 1, :],
                                              o65[64:65, :])
                        ou.append(o65)
                # two batched reciprocals cover all 8 heads of this chunk
                den_rc = [rcpool.tile([128, 512], f32, name=f"dr{j}", tag=f"dr{j}")
                          for j in range(2)]
                for j in range(2):
                    nc.vector.reciprocal(den_rc[j][:], den[j][:])
                wiloc = [opool.tile([128, 512], bf, name=f"wl{p}", tag=f"wl{p}",
                                    bufs=2) for p in range(4)]
                for h in range(HL):
                    r = 32 * (h % 4)
                    r1 = rcpool.tile([1, 512], f32, name="r1", tag="r1")
                    nc.vector.tensor_copy(r1[0:1, :], den_rc[h // 4][r:r + 1, :])
                    bc = bcpool.tile([64, 512], f32, name="bc", tag="bc")
                    nc.gpsimd.partition_broadcast(bc[:], r1[0:1, :])
                    p, po = h // 2, 64 * (h % 2)
                    nc.vector.tensor_tensor(wiloc[p][po:po + 64, :],
                                            ou[h][0:64, :], bc[:], op=ALU.mult)
                for p in range(4):
                    nc.gpsimd.dma_start(ag_in[q][128 * p:128 * p + 128, :],
                                        wiloc[p][:])
                nc.gpsimd.collective_compute(
                    "AllGather", ALU.bypass, replica_groups=groups,
                    ins=[ag_in[q].opt()], outs=[ag_out[q].opt()],
                )

                # ---- output projection for this sq chunk -----------------
                wi = []
                for shard in range(2):
                    t = wipool.tile([128, 4, 512], bf, name="wib", tag="wib",
                                    bufs=3)
                    nc.scalar.dma_start(
                        t[:],
                        ag_out[q][shard].rearrange("(mt p) f -> p mt f", p=128))
                    wi.append(t)
                for so in range(4):
                    ps = psY.tile([128, 512], f32, name="psy", tag="ps_y")
                    for j in range(8):
                        shard, mt = j // 4, j % 4
                        nc.tensor.matmul(
                            ps[:], wi[shard][:, mt, 128 * so:128 * so + 128],
                            wo[j][:],
                            start=(j == 0),
                            stop=(j == 7 and not with_bias),
                        )
                    if with_bias:
                        nc.tensor.matmul(ps[:], ones[0:1, 0:128], bo[0:1, :],
                                         start=False, stop=True)
                    ysb = ypool.tile([128, 512], f32, name="ysb", tag="y")
                    nc.vector.tensor_copy(ysb[:], ps[:])
                    r0 = 512 * q + 128 * so
                    nc.scalar.dma_start(out_d[r0:r0 + 128, :], ysb[:])

    nc.compile()
    return nc


def _get_program(with_bias):
    if with_bias not in _PROGRAM_CACHE:
        _PROGRAM_CACHE[with_bias] = _build_program(with_bias)
    return _PROGRAM_CACHE[with_bias]


def kernel(x, attn_mask, Wq, bq, Wk, bk, Wv, bv, Wo, bo):
    x = np.asarray(x, dtype=np.float32)
    Wq, Wk, Wv, Wo = (np.asarray(w, dtype=np.float32) for w in (Wq, Wk, Wv, Wo))
    bq, bk, bv, bo = (np.asarray(b_, dtype=np.float32) for b_ in (bq, bk, bv, bo))

    with_bias = bool(np.any(bq) or np.any(bk) or np.any(bv) or np.any(bo))
    nc = _get_program(with_bias)

    xT = [np.ascontiguousarray(x[b].T).astype(BF16) for b in range(B)]
    wqT = np.ascontiguousarray(Wq.T).astype(BF16)
    wkT = np.ascontiguousarray(Wk.T).astype(BF16)
    wvT = np.ascontiguousarray(Wv.T).astype(BF16)
    woT = np.ascontiguousarray(Wo.T).astype(BF16)

    pp, ff = np.arange(128)[:, None], np.arange(512)[None, :]
    mmask = np.stack(
        [(pp + 128 * c <= ff).astype(np.float32) for c in range(4)]).astype(BF16)

    in_maps = []
    for c in range(N_CORES):
        b, g = c // 2, c % 2
        sl = slice(DL * g, DL * g + DL)
        m = {
            "xT": xT[b],
            "wqT": np.ascontiguousarray(wqT[:, sl]),
            "wkT": np.ascontiguousarray(wkT[:, sl]),
            "wvT": np.ascontiguousarray(wvT[:, sl]),
            "woT": np.ascontiguousarray(woT[:, sl]),
            "mmask": mmask,
        }
        if with_bias:
            m["bq"] = bq[sl].reshape(1, DL).astype(BF16)
            m["bk"] = bk[sl].reshape(1, DL).astype(BF16)
            m["bv"] = bv[sl].reshape(1, DL).astype(BF16)
            m["bo"] = bo[sl].reshape(1, DL).astype(BF16)
        in_maps.append(m)

    global _LAST_IN_MAPS
    _LAST_IN_MAPS = in_maps
    res = run_bass_kernel_spmd(nc, in_maps, list(range(N_CORES)))

    out = np.empty((B, S, D), dtype=np.float32)
    for b in range(B):
        out[b, :, :DL] = res.results[2 * b]["out"]
        out[b, :, DL:] = res.results[2 * b + 1]["out"]
    return out

